# revision 1
# baseline (speedup 1.0000x reference)
"""FFJORD log-prob loss kernel for Trainium2 (8 NeuronCores, data parallel).

Computes:  -mean(logprob_voxel) - mean(logprob_energy)   (scalar fp32)

Strategy
--------
Pure data parallel over the batch (8192 -> 8 cores x 1024 -> 2 halves x 512).
Everything is kept feature-major ("transposed", [feature, batch]) in SBUF so
that every matmul uses the natural weight matrix as the stationary operand
(lhsT) and no transposes are ever needed on-device.

Math restructuring (exact, not approximate):
 * u1 = eps @ W1[:D]           is constant across all 60 dyn() evals -> once.
 * m3 = eps @ W3.T             lets the Hutchinson trace be computed as
   sum((1-h2^2)*u2 * m3) without the jvp's third matmul -> per eval we do
   4 matmuls instead of 6.
 * Only the batch-MEAN of the trace / logpz is needed, so traces are reduced
   on-chip into [128,1] accumulators and finished on the host.
 * b1 is folded into the L1 matmul via an extra "ones" row of the input;
   t enters via a dedicated partition row of the z buffer; cond rows are
   static rows of the z buffer.
 * dt is folded into the k eviction (k' = dt*k), so stage combinations use
   the raw tableau coefficients.
 * Tangent pass is skipped entirely for stage 1 (b[1] == 0).
 * Matmuls run as float32r (full PE rate); fp32 storage everywhere.
"""

import os
import sys

import numpy as np

sys.path.insert(0, "/opt/trn_rl_repo")

# ----------------------------------------------------------------------------
# Problem constants (hardcoded; kernel.py must be self-contained)
# ----------------------------------------------------------------------------
B_TOT = 8192
N_CORES = 8
BC = B_TOT // N_CORES          # 1024 per core
BH = BC // 2                   # 512 per half (free dim of all on-chip tensors)
D = 504                        # voxel dim
E = 45                         # energy dim
C = 1                          # cond dim
H = 512                        # hidden
N_STEPS = 10
DT = np.float32(-1.0 / N_STEPS)
LOG2PI = float(np.log(2.0 * np.pi))

C_TAB = (0.0, 1 / 5, 3 / 10, 4 / 5, 8 / 9, 1.0)
A_TAB = ((),
         (1 / 5,),
         (3 / 40, 9 / 40),
         (44 / 45, -56 / 15, 32 / 9),
         (19372 / 6561, -25360 / 2187, 64448 / 6561, -212 / 729),
         (9017 / 3168, -355 / 33, 46732 / 5247, 49 / 176, -5103 / 18656))
B_TAB = (35 / 384, 0.0, 500 / 1113, 125 / 192, -2187 / 6784, 11 / 84)

KXV = [128, 128, 128, 120]     # voxel x k-tile / L3-out m-tile partition counts
ZKP = [128, 128, 128, 120, 47]  # voxel L1 k-tile partition counts (x | cond+bias tail)
KIN_E = 48                      # energy L1 k-tile partitions (e,t,cond,ones)

USE_FORI = True

# ----------------------------------------------------------------------------
# Device program
# ----------------------------------------------------------------------------
_CACHE = {}
LAST_RESULTS = None


def _build_program(reps=1):
    import concourse.bass as bass
    import concourse.mybir as mybir
    from concourse import bacc
    from concourse.tile import TileContext

    F32 = mybir.dt.float32
    F32R = mybir.dt.float32r
    ALU = mybir.AluOpType
    AF = mybir.ActivationFunctionType
    ds = bass.ds

    nc = bacc.Bacc(trn_type="TRN2", debug=False)

    dram_in = {}

    def din(name, p, f):
        dram_in[name] = nc.dram_tensor(name, [p, f], F32, kind="ExternalInput").ap()

    # weights / constants
    din("w1v", 128, 5 * 512)
    din("w2v", 128, 4 * 512)
    din("w3v", 128, 4 * 504)
    din("w3vt", 128, 4 * 512)
    din("b2v", 128, 4)
    din("db3v", 128, 4)
    din("w1tg", 128, 4)
    din("w1teg", 128, 4)
    din("w1e", KIN_E, 512)
    din("w2e", 128, 4 * 512)
    din("w3e", 128, 4 * 45)
    din("w3et", 45, 512)
    din("b2e", 128, 4)
    din("db3e", 45, 1)
    din("tg", 128, 60)
    # per-half data
    for h in (0, 1):
        din(f"xv{h}", 128, 4 * BH)
        din(f"ev{h}", 128, 4 * BH)
        din(f"ztl{h}", 47, BH)
        din(f"xe{h}", 45, BH)
        din(f"ee{h}", 45, BH)
        din(f"ce{h}", 3, BH)
    out_d = nc.dram_tensor("out", [128, 14], F32, kind="ExternalOutput").ap()

    HINTS = (mybir.EngineType.PE, mybir.EngineType.DVE,
             mybir.EngineType.Activation, mybir.EngineType.Pool,
             mybir.EngineType.SP)
    W = 4 * BH  # 2048, grouped free width of h-space / x-space tensors
    AW = 3 * BH  # 1536 boundary between full groups and the partial group

    with TileContext(nc) as tc:
        with tc.tile_pool(name="ps", bufs=8, space="PSUM") as ps, \
             tc.tile_pool(name="state", bufs=1) as st:
            T = {}

            def mk(name, p, f, dtype=F32):
                tile = st.tile([p, f], dtype, name=name, tag=name)
                T[name] = tile
                return tile

            # weights (matmul operands carry the float32r dtype so every
            # producer instruction is f32r-tagged, as the BIR verifier wants)
            RSET = {"w1v", "w2v", "w3v", "w3vt", "w1e", "w2e", "w3e", "w3et"}
            for nm, (p, f) in dict(
                w1v=(128, 5 * 512), w2v=(128, 4 * 512), w3v=(128, 4 * 504),
                w3vt=(128, 4 * 512), b2v=(128, 4), db3v=(128, 4),
                w1e=(KIN_E, 512), w2e=(128, 4 * 512), w3e=(128, 4 * 45),
                w3et=(45, 512), b2e=(128, 4), db3e=(45, 1), tg=(128, 60),
                w1tg=(128, 4), w1teg=(128, 4),
            ).items():
                dt_ = F32R if nm in RSET else F32
                mk(nm, p, f, dt_)
                src = dram_in[nm].bitcast(F32R) if nm in RSET else dram_in[nm]
                nc.sync.dma_start(out=T[nm][:, :], in_=src)

            # state (tensors that feed matmuls are float32r-typed)
            xx = mk("xx", 128, W)
            zbufs = [mk("zb0", 128, W, F32R), mk("zb1", 128, W, F32R)]
            ztl = mk("ztl", 47, BH, F32R)
            zes = [mk("ze0", KIN_E, BH, F32R), mk("ze1", KIN_E, BH, F32R)]
            xxe = mk("xxe", 45, BH)
            xacc = mk("xacc", 128, W)
            xacce = mk("xacce", 45, BH)
            kv = [mk(f"kv{j}", 128, W) for j in range(5)]
            ke = [mk(f"ke{j}", 45, BH) for j in range(5)]
            u1v = mk("u1v", 128, W)
            m3v = mk("m3v", 128, W)
            h1v = mk("h1v", 128, W, F32R)
            h2v = mk("h2v", 128, W, F32R)
            u1e = mk("u1e", 128, W)
            m3e = mk("m3e", 128, W)
            h1e = mk("h1e", 128, W, F32R)
            h2e = mk("h2e", 128, W, F32R)
            outs = mk("outs", 128, 14)
            tstep = mk("tstep", 128, 6)
            tbv = mk("tbv", 128, 4)
            tbe = mk("tbe", 128, 4)

            nc.vector.memset(outs[:, :], 0.0)
            for j in range(5):
                # zero the group-3 pad lanes (96..127 rewritten by evicts later)
                nc.vector.memset(kv[j][96:128, AW:W], 0.0)

            w1v, w2v, w3v, w3vt = T["w1v"], T["w2v"], T["w3v"], T["w3vt"]
            w1e, w2e, w3e, w3et = T["w1e"], T["w2e"], T["w3e"], T["w3et"]
            b2v, db3v, b2e, db3e, tg = T["b2v"], T["db3v"], T["b2e"], T["db3e"], T["tg"]
            w1tg, w1teg = T["w1tg"], T["w1teg"]

            def mm(p_out, lhs, rhs, first, last):
                nc.tensor.matmul(p_out, lhs.bitcast(F32R), rhs.bitcast(F32R),
                                 start=first, stop=last)

            def prologue(half):
                nc.sync.dma_start(out=xx[:, :], in_=dram_in[f"xv{half}"])
                nc.sync.dma_start(out=h1v[:, :],
                                  in_=dram_in[f"ev{half}"].bitcast(F32R))  # eps_v
                nc.gpsimd.dma_start(out=ztl[:, :],
                                    in_=dram_in[f"ztl{half}"].bitcast(F32R))
                nc.gpsimd.dma_start(out=xxe[:, :], in_=dram_in[f"xe{half}"])
                nc.gpsimd.dma_start(out=h1e[0:45, 0:BH],
                                    in_=dram_in[f"ee{half}"].bitcast(F32R))  # eps_e
                nc.gpsimd.dma_start(out=zes[0][45:48, :],
                                    in_=dram_in[f"ce{half}"].bitcast(F32R))
                nc.gpsimd.dma_start(out=zes[1][45:48, :],
                                    in_=dram_in[f"ce{half}"].bitcast(F32R))
                # u1v / m3v / u1e / m3e
                for m in range(4):
                    mb = slice(m * BH, (m + 1) * BH)
                    p = ps.tile([128, BH], F32, tag="ps", name="pp1")
                    for g in range(4):
                        kp = KXV[g]
                        mm(p[:, :], w1v[0:kp, g * 512 + m * 128:g * 512 + (m + 1) * 128],
                           h1v[0:kp, g * BH:(g + 1) * BH], g == 0, g == 3)
                    nc.scalar.activation(u1v[:, mb], p[:, :], AF.Copy)
                    p = ps.tile([128, BH], F32, tag="ps", name="pp2")
                    for g in range(4):
                        kp = KXV[g]
                        mm(p[:, :], w3vt[0:kp, g * 512 + m * 128:g * 512 + (m + 1) * 128],
                           h1v[0:kp, g * BH:(g + 1) * BH], g == 0, g == 3)
                    nc.scalar.activation(m3v[:, mb], p[:, :], AF.Copy)
                    p = ps.tile([128, BH], F32, tag="ps", name="pp3")
                    mm(p[:, :], w1e[0:45, m * 128:(m + 1) * 128], h1e[0:45, 0:BH],
                       True, True)
                    nc.scalar.activation(u1e[:, mb], p[:, :], AF.Copy)
                    p = ps.tile([128, BH], F32, tag="ps", name="pp4")
                    mm(p[:, :], w3et[0:45, m * 128:(m + 1) * 128], h1e[0:45, 0:BH],
                       True, True)
                    nc.scalar.activation(m3e[:, mb], p[:, :], AF.Copy)

            def stage(half, iv, s, col_tv, col_te, col_qv, col_qe):
                zb = zbufs[s % 2]
                ze = zes[s % 2]
                # ---- stage input build ----
                if s == 0:
                    nc.gpsimd.tensor_copy(out=zb[:, 0:W], in_=xx[:, 0:W])
                    nc.gpsimd.tensor_copy(out=ze[0:45, :], in_=xxe[0:45, :])
                else:
                    a = A_TAB[s]
                    nc.vector.scalar_tensor_tensor(
                        zb[:, 0:W], kv[0][:, 0:W], float(a[0]), xx[:, 0:W],
                        ALU.mult, ALU.add)
                    nc.vector.scalar_tensor_tensor(
                        ze[0:45, :], ke[0][0:45, :], float(a[0]), xxe[0:45, :],
                        ALU.mult, ALU.add)
                    for j in range(1, s):
                        nc.vector.scalar_tensor_tensor(
                            zb[:, 0:W], kv[j][:, 0:W], float(a[j]), zb[:, 0:W],
                            ALU.mult, ALU.add)
                        nc.vector.scalar_tensor_tensor(
                            ze[0:45, :], ke[j][0:45, :], float(a[j]), ze[0:45, :],
                            ALU.mult, ALU.add)
                # ---- t enters layer 1 via the tanh bias: tb = t * W1[t_row] ----
                tsl = slice(s, s + 1)
                nc.vector.tensor_scalar(tbv[:, :], w1tg[:, :], tstep[:, tsl],
                                        None, ALU.mult)
                nc.vector.tensor_scalar(tbe[:, :], w1teg[:, :], tstep[:, tsl],
                                        None, ALU.mult)

                # ---- L1 + tanh ----
                for m in range(4):
                    mb = slice(m * BH, (m + 1) * BH)
                    p = ps.tile([128, BH], F32, tag="ps", name="pv1")
                    for g in range(5):
                        lhs = w1v[0:ZKP[g], g * 512 + m * 128:g * 512 + (m + 1) * 128]
                        rhs = (zb[0:ZKP[g], g * BH:(g + 1) * BH] if g < 4
                               else ztl[0:47, :])
                        mm(p[:, :], lhs, rhs, g == 0, g == 4)
                    nc.scalar.activation(h1v[:, mb], p[:, :], AF.Tanh,
                                         bias=tbv[:, m:m + 1])
                for m in range(4):
                    mb = slice(m * BH, (m + 1) * BH)
                    p = ps.tile([128, BH], F32, tag="ps", name="pe1")
                    mm(p[:, :], w1e[0:KIN_E, m * 128:(m + 1) * 128], ze[0:KIN_E, :],
                       True, True)
                    nc.scalar.activation(h1e[:, mb], p[:, :], AF.Tanh,
                                         bias=tbe[:, m:m + 1])
                # ---- L2 + tanh ----
                for m in range(4):
                    mb = slice(m * BH, (m + 1) * BH)
                    p = ps.tile([128, BH], F32, tag="ps", name="pv2")
                    for g in range(4):
                        mm(p[:, :], w2v[:, g * 512 + m * 128:g * 512 + (m + 1) * 128],
                           h1v[:, g * BH:(g + 1) * BH], g == 0, g == 3)
                    nc.scalar.activation(h2v[:, mb], p[:, :], AF.Tanh,
                                         bias=b2v[:, m:m + 1])
                for m in range(4):
                    mb = slice(m * BH, (m + 1) * BH)
                    p = ps.tile([128, BH], F32, tag="ps", name="pe2")
                    for g in range(4):
                        mm(p[:, :], w2e[:, g * 512 + m * 128:g * 512 + (m + 1) * 128],
                           h1e[:, g * BH:(g + 1) * BH], g == 0, g == 3)
                    nc.scalar.activation(h2e[:, mb], p[:, :], AF.Tanh,
                                         bias=b2e[:, m:m + 1])
                # ---- L3 + evict (k' = dt*k, bias dt*b3) ----
                kvdst = zb if s == 5 else kv[s]
                for m in range(4):
                    mp = KXV[m]
                    p = ps.tile([128, BH], F32, tag="ps", name="pv3")
                    for g in range(4):
                        mm(p[0:mp, :], w3v[:, g * 504 + m * 128:g * 504 + m * 128 + mp],
                           h2v[:, g * BH:(g + 1) * BH], g == 0, g == 3)
                    nc.scalar.activation(kvdst[0:mp, m * BH:(m + 1) * BH], p[0:mp, :],
                                         AF.Identity, bias=db3v[0:mp, m:m + 1],
                                         scale=float(DT))
                kedst = ze if s == 5 else ke[s]
                p = ps.tile([128, BH], F32, tag="ps", name="pe3")
                for g in range(4):
                    mm(p[0:45, :], w3e[:, g * 45:(g + 1) * 45],
                       h2e[:, g * BH:(g + 1) * BH], g == 0, g == 3)
                nc.scalar.activation(kedst[0:45, :], p[0:45, :], AF.Identity,
                                     bias=db3e[0:45, 0:1], scale=float(DT))

                # ---- tangent (only when this stage's trace matters) ----
                if B_TAB[s] != 0.0:
                    ttr_scale = float(DT) * float(B_TAB[s])
                    # voxel: h1 <- h1^2 <- (h1^2-1)*u1 (= -g1), u2, g2, trace
                    nc.scalar.activation(h1v[:, :], h1v[:, :], AF.Square)
                    nc.vector.scalar_tensor_tensor(h1v[:, :], h1v[:, :], 1.0,
                                                   u1v[:, :], ALU.subtract, ALU.mult)
                    u2p = []
                    for m in range(4):
                        p = ps.tile([128, BH], F32, tag="ps", name="pv4")
                        u2p.append(p)
                        for g in range(4):
                            mm(p[:, :],
                               w2v[:, g * 512 + m * 128:g * 512 + (m + 1) * 128],
                               h1v[:, g * BH:(g + 1) * BH], g == 0, g == 3)
                    nc.scalar.activation(h2v[:, :], h2v[:, :], AF.Square)
                    for m in range(4):
                        mb = slice(m * BH, (m + 1) * BH)
                        nc.vector.scalar_tensor_tensor(h2v[:, mb], h2v[:, mb], 1.0,
                                                       u2p[m][:, :], ALU.subtract,
                                                       ALU.mult)
                    nc.vector.scalar_tensor_tensor(
                        h2v[:, :], h2v[:, :], 1.0, m3v[:, :],
                        ALU.mult, ALU.mult, accum_out=outs[:, col_qv:col_qv + 1])
                    nc.vector.scalar_tensor_tensor(
                        outs[:, col_tv:col_tv + 1], outs[:, col_qv:col_qv + 1],
                        ttr_scale, outs[:, col_tv:col_tv + 1], ALU.mult, ALU.add)
                    # energy
                    nc.scalar.activation(h1e[:, :], h1e[:, :], AF.Square)
                    nc.vector.scalar_tensor_tensor(h1e[:, :], h1e[:, :], 1.0,
                                                   u1e[:, :], ALU.subtract, ALU.mult)
                    u2pe = []
                    for m in range(4):
                        p = ps.tile([128, BH], F32, tag="ps", name="pe4")
                        u2pe.append(p)
                        for g in range(4):
                            mm(p[:, :],
                               w2e[:, g * 512 + m * 128:g * 512 + (m + 1) * 128],
                               h1e[:, g * BH:(g + 1) * BH], g == 0, g == 3)
                    nc.scalar.activation(h2e[:, :], h2e[:, :], AF.Square)
                    for m in range(4):
                        mb = slice(m * BH, (m + 1) * BH)
                        nc.vector.scalar_tensor_tensor(h2e[:, mb], h2e[:, mb], 1.0,
                                                       u2pe[m][:, :], ALU.subtract,
                                                       ALU.mult)
                    nc.vector.scalar_tensor_tensor(
                        h2e[:, :], h2e[:, :], 1.0, m3e[:, :],
                        ALU.mult, ALU.mult, accum_out=outs[:, col_qe:col_qe + 1])
                    nc.vector.scalar_tensor_tensor(
                        outs[:, col_te:col_te + 1], outs[:, col_qe:col_qe + 1],
                        ttr_scale, outs[:, col_te:col_te + 1], ALU.mult, ALU.add)

            def step_body(half, iv, col_tv, col_te, col_qv, col_qe):
                if USE_FORI:
                    nc.vector.tensor_copy(out=tstep[:, :], in_=tg[:, ds(iv * 6, 6)])
                else:
                    i0 = iv * 6
                    nc.vector.tensor_copy(out=tstep[:, :], in_=tg[:, i0:i0 + 6])
                for s in range(6):
                    stage(half, iv, s, col_tv, col_te, col_qv, col_qe)
                    # accumulate the final-update term for k_s as soon as it
                    # exists (keeps the step tail off the critical path)
                    if s == 0:
                        nc.vector.tensor_scalar_mul(xacc[:, 0:W], kv[0][:, 0:W],
                                                    float(B_TAB[0]))
                        nc.vector.tensor_scalar_mul(xacce[0:45, :], ke[0][0:45, :],
                                                    float(B_TAB[0]))
                    elif s in (2, 3, 4):
                        bj = float(B_TAB[s])
                        nc.vector.scalar_tensor_tensor(
                            xacc[:, 0:W], kv[s][:, 0:W], bj, xacc[:, 0:W],
                            ALU.mult, ALU.add)
                        nc.vector.scalar_tensor_tensor(
                            xacce[0:45, :], ke[s][0:45, :], bj, xacce[0:45, :],
                            ALU.mult, ALU.add)
                # tail: xx += xacc + b5*k5   (k'_5 lives in zb1/ze1)
                b5 = float(B_TAB[5])
                nc.vector.scalar_tensor_tensor(
                    xacc[:, 0:W], zbufs[1][:, 0:W], b5, xacc[:, 0:W],
                    ALU.mult, ALU.add)
                nc.vector.tensor_add(out=xx[:, 0:W], in0=xx[:, 0:W],
                                     in1=xacc[:, 0:W])
                nc.vector.scalar_tensor_tensor(
                    xacce[0:45, :], zes[1][0:45, :], b5, xacce[0:45, :],
                    ALU.mult, ALU.add)
                nc.vector.tensor_add(out=xxe[0:45, :], in0=xxe[0:45, :],
                                     in1=xacce[0:45, :])

            def epilogue(half, col_zvA, col_zvB, col_ze):
                nc.scalar.activation(kv[0][:, 0:W], xx[:, 0:W], AF.Square,
                                     accum_out=outs[:, col_zvA:col_zvA + 1])
                nc.scalar.activation(ke[0][0:45, 0:BH], xxe[0:45, :], AF.Square,
                                     accum_out=outs[0:45, col_ze:col_ze + 1])

            def whole_pass():
                nc.vector.memset(outs[:, :], 0.0)
                for half in (0, 1):
                    c0 = half * 7
                    tc.strict_bb_all_engine_barrier()
                    prologue(half)
                    if USE_FORI:
                        with tc.For_i(0, N_STEPS, hint_engines=HINTS) as iv:
                            step_body(half, iv, c0 + 0, c0 + 1, c0 + 5, c0 + 6)
                    else:
                        for iv in range(N_STEPS):
                            step_body(half, iv, c0 + 0, c0 + 1, c0 + 5, c0 + 6)
                    epilogue(half, c0 + 2, c0 + 3, c0 + 4)

            if reps == 1:
                whole_pass()
            else:
                with tc.For_i(0, reps, hint_engines=HINTS):
                    whole_pass()

            nc.sync.dma_start(out=out_d, in_=outs[:, :])

    nc.compile()
    return nc


def _get_program(reps=1):
    key = f"nc{reps}"
    if key not in _CACHE:
        _CACHE[key] = _build_program(reps)
    return _CACHE[key]


# ----------------------------------------------------------------------------
# Host-side packing
# ----------------------------------------------------------------------------
def _group_feat(xT, ngroups, rows_total):
    """[F, Bh] feature-major -> [128, ngroups*Bh] grouped, zero padded."""
    F, Bh = xT.shape
    assert F == rows_total
    out = np.zeros((128, ngroups * Bh), np.float32)
    for g in range(ngroups):
        r0, r1 = g * 128, min((g + 1) * 128, F)
        if r0 >= F:
            break
        out[0:r1 - r0, g * Bh:g * Bh + Bh] = xT[r0:r1]
    return out


def _pack_weights(inputs):
    W1v = np.asarray(inputs["W1v"], np.float32)
    b1v = np.asarray(inputs["b1v"], np.float32)
    W2v = np.asarray(inputs["W2v"], np.float32)
    b2v = np.asarray(inputs["b2v"], np.float32)
    W3v = np.asarray(inputs["W3v"], np.float32)
    b3v = np.asarray(inputs["b3v"], np.float32)
    W1e = np.asarray(inputs["W1e"], np.float32)
    b1e = np.asarray(inputs["b1e"], np.float32)
    W2e = np.asarray(inputs["W2e"], np.float32)
    b2e = np.asarray(inputs["b2e"], np.float32)
    W3e = np.asarray(inputs["W3e"], np.float32)
    b3e = np.asarray(inputs["b3e"], np.float32)

    d = {}
    # k-tiles g0..g3 = x rows (504, zero padded), tail tile = [cond rows; b1]
    w1x = _group_feat(W1v[:D], 4, D)                # [128, 4*512]
    tail = np.zeros((128, 512), np.float32)
    tail[0:46] = W1v[D + 1:D + 47]                  # cond rows
    tail[46] = b1v
    d["w1v"] = np.concatenate([w1x, tail], axis=1)  # [128, 5*512]
    d["w2v"] = _group_feat(W2v, 4, 512)
    d["w3v"] = _group_feat(W3v, 4, 512)             # [128, 4*504]
    d["w3vt"] = _group_feat(np.ascontiguousarray(W3v.T), 4, 504)
    d["b2v"] = np.ascontiguousarray(b2v.reshape(4, 128).T)
    db3 = (DT * b3v).astype(np.float32)
    db3g = np.zeros((128, 4), np.float32)
    for m in range(4):
        r0, r1 = m * 128, min((m + 1) * 128, 504)
        db3g[0:r1 - r0, m] = db3[r0:r1]
    d["db3v"] = db3g
    d["w1tg"] = np.ascontiguousarray(W1v[D].reshape(4, 128).T)   # t row of W1v
    w1eaug = np.vstack([W1e, b1e[None, :]])         # [48, 512]
    d["w1e"] = np.ascontiguousarray(w1eaug)
    d["w2e"] = _group_feat(W2e, 4, 512)
    d["w3e"] = _group_feat(W3e, 4, 512)             # [128, 4*45]
    d["w3et"] = np.ascontiguousarray(W3e.T)         # [45, 512]
    d["b2e"] = np.ascontiguousarray(b2e.reshape(4, 128).T)
    d["db3e"] = (DT * b3e).astype(np.float32)[:, None]
    d["w1teg"] = np.ascontiguousarray(W1e[E].reshape(4, 128).T)  # t row of W1e
    tv = np.zeros(60, np.float32)
    for n in range(N_STEPS):
        t0 = np.float32(1.0) + DT * np.float32(n)
        for s in range(6):
            tv[6 * n + s] = t0 + np.float32(C_TAB[s]) * DT
    d["tg"] = np.tile(tv[None, :], (128, 1)).astype(np.float32)
    return d


def _pack_core(inputs, wpack, core):
    voxel = np.asarray(inputs["voxel"], np.float32)[core * BC:(core + 1) * BC]
    energy = np.asarray(inputs["energy"], np.float32)[core * BC:(core + 1) * BC]
    cond = np.asarray(inputs["cond"], np.float32)[core * BC:(core + 1) * BC]
    eps_v = np.asarray(inputs["eps_v"], np.float32)[core * BC:(core + 1) * BC]
    eps_e = np.asarray(inputs["eps_e"], np.float32)[core * BC:(core + 1) * BC]

    m = dict(wpack)
    for h in (0, 1):
        sl = slice(h * BH, (h + 1) * BH)
        xT = np.ascontiguousarray(voxel[sl].T)       # [504, 512]
        evT = np.ascontiguousarray(eps_v[sl].T)
        m[f"xv{h}"] = _group_feat(xT, 4, D)
        m[f"ev{h}"] = _group_feat(evT, 4, D)
        condv = np.ascontiguousarray(
            np.concatenate([energy[sl], cond[sl]], axis=1).T)  # [46, 512]
        ztl = np.ones((47, BH), np.float32)
        ztl[0:46] = condv
        m[f"ztl{h}"] = ztl
        m[f"xe{h}"] = np.ascontiguousarray(energy[sl].T)
        m[f"ee{h}"] = np.ascontiguousarray(eps_e[sl].T)
        ce = np.ones((3, BH), np.float32)
        ce[0] = 0.0            # t lane (t enters via the tanh bias)
        ce[1] = cond[sl, 0]
        m[f"ce{h}"] = ce
    return m


# ----------------------------------------------------------------------------
# Entry point
# ----------------------------------------------------------------------------
def kernel(**inputs) -> np.ndarray:
    global LAST_RESULTS
    from concourse import bass_utils

    nc = _get_program()
    wpack = _pack_weights(inputs)
    in_maps = [_pack_core(inputs, wpack, c) for c in range(N_CORES)]
    res = bass_utils.run_bass_kernel_spmd(nc, in_maps, core_ids=list(range(N_CORES)))
    LAST_RESULTS = res

    total = np.zeros((128, 14), np.float64)
    for r in res.results:
        total += r["out"].astype(np.float64)
    trv = total[:, 0].sum() + total[:, 7].sum()
    tre = total[:, 1].sum() + total[:, 8].sum()
    zsv = (total[:, 2].sum() + total[:, 3].sum()
           + total[:, 9].sum() + total[:, 10].sum())
    zse = total[:, 4].sum() + total[:, 11].sum()

    mean_lp_v = (-0.5 * zsv + trv) / B_TOT - 0.5 * D * LOG2PI
    mean_lp_e = (-0.5 * zse + tre) / B_TOT - 0.5 * E * LOG2PI
    loss = -(mean_lp_v + mean_lp_e)
    return np.array(loss, dtype=np.float32)



# revision 17
# speedup vs baseline: 1.2649x; 1.2649x over previous
"""FFJORD log-prob loss kernel for Trainium2 (8 NeuronCores, data parallel).

Computes:  -mean(logprob_voxel) - mean(logprob_energy)   (scalar fp32)

Strategy
--------
Pure data parallel over the batch (8192 -> 8 cores x 1024 -> 2 halves x 512).
Everything is kept feature-major ("transposed", [feature, batch]) in SBUF so
that every matmul uses the natural weight matrix as the stationary operand
(lhsT) and no transposes are ever needed on-device.

Math restructuring (exact, not approximate):
 * u1 = eps @ W1[:D]           is constant across all 60 dyn() evals -> once.
 * m3 = eps @ W3.T             lets the Hutchinson trace be computed as
   sum((1-h2^2)*u2 * m3) without the jvp's third matmul -> per eval we do
   4 matmuls instead of 6.
 * Only the batch-MEAN of the trace / logpz is needed, so traces are reduced
   on-chip into [128,1] accumulators and finished on the host.
 * b1 is folded into the L1 matmul via an extra "ones" row of the input;
   t enters via a dedicated partition row of the z buffer; cond rows are
   static rows of the z buffer.
 * dt is folded into the k eviction (k' = dt*k), so stage combinations use
   the raw tableau coefficients.
 * Tangent pass is skipped entirely for stage 1 (b[1] == 0).
 * Matmuls run as float32r (full PE rate); fp32 storage everywhere.
"""

import os
import sys

import numpy as np

sys.path.insert(0, "/opt/trn_rl_repo")

# ----------------------------------------------------------------------------
# Problem constants (hardcoded; kernel.py must be self-contained)
# ----------------------------------------------------------------------------
B_TOT = 8192
N_CORES = 8
BC = B_TOT // N_CORES          # 1024 per core
BH = BC // 2                   # 512 per half (free dim of all on-chip tensors)
D = 504                        # voxel dim
E = 45                         # energy dim
C = 1                          # cond dim
H = 512                        # hidden
N_STEPS = 10
DT = np.float32(-1.0 / N_STEPS)
LOG2PI = float(np.log(2.0 * np.pi))

C_TAB = (0.0, 1 / 5, 3 / 10, 4 / 5, 8 / 9, 1.0)
A_TAB = ((),
         (1 / 5,),
         (3 / 40, 9 / 40),
         (44 / 45, -56 / 15, 32 / 9),
         (19372 / 6561, -25360 / 2187, 64448 / 6561, -212 / 729),
         (9017 / 3168, -355 / 33, 46732 / 5247, 49 / 176, -5103 / 18656))
B_TAB = (35 / 384, 0.0, 500 / 1113, 125 / 192, -2187 / 6784, 11 / 84)

KXV = [128, 128, 128, 120]     # voxel x k-tile / L3-out m-tile partition counts
ZKP = [128, 128, 128, 120, 47]  # voxel L1 k-tile partition counts (x | cond+bias tail)
KIN_E = 48                      # energy L1 k-tile partitions (e,t,cond,ones)

W2SCALE = 16.0                 # fp8 W2 stored x16 (undone at tanh / via m3)
W3SCALE = 2048.0               # fp8 W3 stored x(dt*2048) (undone at eviction)

USE_FORI = True

# ----------------------------------------------------------------------------
# Device program
# ----------------------------------------------------------------------------
_CACHE = {}
LAST_RESULTS = None


def _build_program(reps=1):
    import concourse.bass as bass
    import concourse.mybir as mybir
    from concourse import bacc
    from concourse.tile import TileContext

    F32 = mybir.dt.float32
    F32R = mybir.dt.float32r
    F8 = mybir.dt.float8e4
    DRM = mybir.MatmulPerfMode.DoubleRow
    ALU = mybir.AluOpType
    AF = mybir.ActivationFunctionType
    ds = bass.ds

    nc = bacc.Bacc(trn_type="TRN2", debug=False)

    dram_in = {}

    def din(name, shape, dtype=F32):
        dram_in[name] = nc.dram_tensor(name, list(shape), dtype,
                                       kind="ExternalInput").ap()

    # weights / constants (fp8 tensors carry host-side scales, see packing)
    # fp8 DR operands are 4-D [128, pair-group, 2, inner] per the s3_lw
    # dual-fp8 ISA shape (pair dim must be AP dim 2, inner step % 16 == 0)
    din("w1v", (128, 5 * 512))
    din("w2v", (128, 2, 2, 512), F8)
    din("w3v", (128, 2, 2, 512), F8)
    din("w3vt", (128, 4 * 512))
    din("b2v", (128, 4))
    din("db3v", (128, 4))
    din("w1tg", (128, 4))
    din("w1teg", (128, 4))
    din("w1e", (KIN_E, 512))
    din("w2e", (128, 2, 2, 512), F8)
    din("w3e", (128, 2, 2, 48), F8)
    din("w3et", (45, 512))
    din("b2e", (128, 4))
    din("db3e", (45, 1))
    din("tg", (128, 60))
    # per-half data
    for h in (0, 1):
        din(f"xv{h}", (128, 4 * BH))
        din(f"ev{h}", (128, 4 * BH))
        din(f"ztl{h}", (47, BH))
        din(f"xe{h}", (45, BH))
        din(f"ee{h}", (45, BH))
        din(f"ce{h}", (3, BH))
    out_d = nc.dram_tensor("out", [128, 14], F32, kind="ExternalOutput").ap()

    HINTS = (mybir.EngineType.PE, mybir.EngineType.DVE,
             mybir.EngineType.Activation, mybir.EngineType.Pool,
             mybir.EngineType.SP)
    W = 4 * BH  # 2048, grouped free width of h-space / x-space tensors
    AW = 3 * BH  # 1536 boundary between full groups and the partial group

    with TileContext(nc) as tc:
        with tc.tile_pool(name="ps", bufs=8, space="PSUM") as ps, \
             tc.tile_pool(name="state", bufs=1) as st:
            T = {}

            def mk(name, *shape, dtype=F32):
                tile = st.tile(list(shape), dtype, name=name, tag=name)
                T[name] = tile
                return tile

            # weights (f32 matmul operands carry the float32r dtype so every
            # producer instruction is f32r-tagged, as the BIR verifier wants;
            # the big h-space weights are fp8 for DoubleRow matmuls)
            RSET = {"w1v", "w3vt", "w1e", "w3et"}
            F8SET = {"w2v", "w3v", "w2e", "w3e"}
            for nm, shp in dict(
                w1v=(128, 5 * 512), w2v=(128, 2, 2, 512), w3v=(128, 2, 2, 512),
                w3vt=(128, 4 * 512), b2v=(128, 4), db3v=(128, 4),
                w1e=(KIN_E, 512), w2e=(128, 2, 2, 512), w3e=(128, 2, 2, 48),
                w3et=(45, 512), b2e=(128, 4), db3e=(45, 1), tg=(128, 60),
                w1tg=(128, 4), w1teg=(128, 4),
            ).items():
                dt_ = F32R if nm in RSET else (F8 if nm in F8SET else F32)
                mk(nm, *shp, dtype=dt_)
                src = dram_in[nm].bitcast(F32R) if nm in RSET else dram_in[nm]
                dst = T[nm][tuple(slice(None) for _ in shp)]
                nc.sync.dma_start(out=dst, in_=src)

            # state (tensors that feed matmuls are float32r/fp8-typed)
            xx = mk("xx", 128, W)
            zbufs = [mk("zb0", 128, W, dtype=F32R), mk("zb1", 128, W, dtype=F32R)]
            ztl = mk("ztl", 47, BH, dtype=F32R)
            zes = [mk("ze0", KIN_E, BH, dtype=F32R), mk("ze1", KIN_E, BH, dtype=F32R)]
            xxe = mk("xxe", 45, BH)
            xacc = mk("xacc", 128, W)
            xacce = mk("xacce", 45, BH)
            kv = [mk(f"kv{j}", 128, W) for j in range(5)]
            ke = [mk(f"ke{j}", 45, BH) for j in range(5)]
            u1v = mk("u1v", 128, W)
            m3v = mk("m3v", 128, W)
            epv = mk("epv", 128, W, dtype=F32R)
            epe = mk("epe", 45, BH, dtype=F32R)
            h1v = mk("h1v", 128, 2, 2, BH, dtype=F8)
            h2v = mk("h2v", 128, 2, 2, BH, dtype=F8)
            u1e = mk("u1e", 128, W)
            m3e = mk("m3e", 128, W)
            h1e = mk("h1e", 128, 2, 2, BH, dtype=F8)
            h2e = mk("h2e", 128, 2, 2, BH, dtype=F8)
            outs = mk("outs", 128, 14)
            tstep = mk("tstep", 128, 6)
            tbv = mk("tbv", 128, 4)
            tbe = mk("tbe", 128, 4)

            nc.vector.memset(outs[:, :], 0.0)
            for j in range(5):
                # zero the group-3 pad lanes (96..127 rewritten by evicts later)
                nc.vector.memset(kv[j][96:128, AW:W], 0.0)

            w1v, w2v, w3v, w3vt = T["w1v"], T["w2v"], T["w3v"], T["w3vt"]
            w1e, w2e, w3e, w3et = T["w1e"], T["w2e"], T["w3e"], T["w3et"]
            b2v, db3v, b2e, db3e, tg = T["b2v"], T["db3v"], T["b2e"], T["db3e"], T["tg"]
            w1tg, w1teg = T["w1tg"], T["w1teg"]

            def mm(p_out, lhs, rhs, first, last):
                nc.tensor.matmul(p_out, lhs.bitcast(F32R), rhs.bitcast(F32R),
                                 start=first, stop=last)

            def mmdr(p_out, lhs, rhs, first, last):
                # fp8 DoubleRow: lhs [128, 2, M], rhs [128, 2, N], K=256/instr
                nc.tensor.matmul(p_out, lhs, rhs, start=first, stop=last,
                                 perf_mode=DRM)

            def prologue(half):
                nc.sync.dma_start(out=xx[:, :], in_=dram_in[f"xv{half}"])
                nc.sync.dma_start(out=epv[:, :],
                                  in_=dram_in[f"ev{half}"].bitcast(F32R))  # eps_v
                nc.gpsimd.dma_start(out=ztl[:, :],
                                    in_=dram_in[f"ztl{half}"].bitcast(F32R))
                nc.gpsimd.dma_start(out=xxe[:, :], in_=dram_in[f"xe{half}"])
                nc.gpsimd.dma_start(out=epe[0:45, 0:BH],
                                    in_=dram_in[f"ee{half}"].bitcast(F32R))  # eps_e
                nc.gpsimd.dma_start(out=zes[0][45:48, :],
                                    in_=dram_in[f"ce{half}"].bitcast(F32R))
                nc.gpsimd.dma_start(out=zes[1][45:48, :],
                                    in_=dram_in[f"ce{half}"].bitcast(F32R))
                # u1v / m3v / u1e / m3e  (m3 carries a 1/16 scale that cancels
                # the x16 on the fp8 W2 when the trace is formed)
                for m in range(4):
                    mb = slice(m * BH, (m + 1) * BH)
                    p = ps.tile([128, BH], F32, tag="ps", name="pp1")
                    for g in range(4):
                        kp = KXV[g]
                        mm(p[:, :], w1v[0:kp, g * 512 + m * 128:g * 512 + (m + 1) * 128],
                           epv[0:kp, g * BH:(g + 1) * BH], g == 0, g == 3)
                    nc.scalar.activation(u1v[:, mb], p[:, :], AF.Copy)
                    p = ps.tile([128, BH], F32, tag="ps", name="pp2")
                    for g in range(4):
                        kp = KXV[g]
                        mm(p[:, :], w3vt[0:kp, g * 512 + m * 128:g * 512 + (m + 1) * 128],
                           epv[0:kp, g * BH:(g + 1) * BH], g == 0, g == 3)
                    nc.scalar.activation(m3v[:, mb], p[:, :], AF.Copy,
                                         scale=1.0 / W2SCALE)
                    p = ps.tile([128, BH], F32, tag="ps", name="pp3")
                    mm(p[:, :], w1e[0:45, m * 128:(m + 1) * 128], epe[0:45, 0:BH],
                       True, True)
                    nc.scalar.activation(u1e[:, mb], p[:, :], AF.Copy)
                    p = ps.tile([128, BH], F32, tag="ps", name="pp4")
                    mm(p[:, :], w3et[0:45, m * 128:(m + 1) * 128], epe[0:45, 0:BH],
                       True, True)
                    nc.scalar.activation(m3e[:, mb], p[:, :], AF.Copy,
                                         scale=1.0 / W2SCALE)

            def stage(half, iv, s, col_tv, col_te, col_qv, col_qe):
                zb = zbufs[s % 2]
                ze = zes[s % 2]
                # ---- stage input build ----
                if s == 0:
                    nc.gpsimd.tensor_copy(out=zb[:, 0:W], in_=xx[:, 0:W])
                    nc.gpsimd.tensor_copy(out=ze[0:45, :], in_=xxe[0:45, :])
                else:
                    a = A_TAB[s]
                    nc.vector.scalar_tensor_tensor(
                        zb[:, 0:W], kv[0][:, 0:W], float(a[0]), xx[:, 0:W],
                        ALU.mult, ALU.add)
                    nc.vector.scalar_tensor_tensor(
                        ze[0:45, :], ke[0][0:45, :], float(a[0]), xxe[0:45, :],
                        ALU.mult, ALU.add)
                    for j in range(1, s):
                        nc.vector.scalar_tensor_tensor(
                            zb[:, 0:W], kv[j][:, 0:W], float(a[j]), zb[:, 0:W],
                            ALU.mult, ALU.add)
                        nc.vector.scalar_tensor_tensor(
                            ze[0:45, :], ke[j][0:45, :], float(a[j]), ze[0:45, :],
                            ALU.mult, ALU.add)
                # ---- t enters layer 1 via the tanh bias: tb = t * W1[t_row] ----
                tsl = slice(s, s + 1)
                nc.vector.tensor_scalar(tbv[:, :], w1tg[:, :], tstep[:, tsl],
                                        None, ALU.mult)
                nc.vector.tensor_scalar(tbe[:, :], w1teg[:, :], tstep[:, tsl],
                                        None, ALU.mult)

                # ---- L1 + tanh (fp32r matmul, fp8 eviction) ----
                for m in range(4):
                    p = ps.tile([128, BH], F32, tag="ps", name="pv1")
                    for g in range(5):
                        lhs = w1v[0:ZKP[g], g * 512 + m * 128:g * 512 + (m + 1) * 128]
                        rhs = (zb[0:ZKP[g], g * BH:(g + 1) * BH] if g < 4
                               else ztl[0:47, :])
                        mm(p[:, :], lhs, rhs, g == 0, g == 4)
                    nc.scalar.activation(h1v[:, m // 2, m % 2, :], p[:, :], AF.Tanh,
                                         bias=tbv[:, m:m + 1])
                for m in range(4):
                    p = ps.tile([128, BH], F32, tag="ps", name="pe1")
                    mm(p[:, :], w1e[0:KIN_E, m * 128:(m + 1) * 128], ze[0:KIN_E, :],
                       True, True)
                    nc.scalar.activation(h1e[:, m // 2, m % 2, :], p[:, :], AF.Tanh,
                                         bias=tbe[:, m:m + 1])
                # ---- L2 + tanh (fp8 DoubleRow; weights carry x16) ----
                for m in range(4):
                    ms = slice(m * 128, (m + 1) * 128)
                    p = ps.tile([128, BH], F32, tag="ps", name="pv2")
                    for g in range(2):
                        mmdr(p[:, :], w2v[:, g, :, ms],
                             h1v[:, g, :, :], g == 0, g == 1)
                    nc.scalar.activation(h2v[:, m // 2, m % 2, :], p[:, :], AF.Tanh,
                                         bias=b2v[:, m:m + 1], scale=1.0 / W2SCALE)
                for m in range(4):
                    ms = slice(m * 128, (m + 1) * 128)
                    p = ps.tile([128, BH], F32, tag="ps", name="pe2")
                    for g in range(2):
                        mmdr(p[:, :], w2e[:, g, :, ms],
                             h1e[:, g, :, :], g == 0, g == 1)
                    nc.scalar.activation(h2e[:, m // 2, m % 2, :], p[:, :], AF.Tanh,
                                         bias=b2e[:, m:m + 1], scale=1.0 / W2SCALE)
                # ---- L3 + evict (weights carry dt and x2048) ----
                kvdst = zb if s == 5 else kv[s]
                for m in range(4):
                    mp = KXV[m]
                    p = ps.tile([128, BH], F32, tag="ps", name="pv3")
                    for g in range(2):
                        mmdr(p[0:mp, :],
                             w3v[:, g, :, m * 128:m * 128 + mp],
                             h2v[:, g, :, :], g == 0, g == 1)
                    nc.scalar.activation(kvdst[0:mp, m * BH:(m + 1) * BH], p[0:mp, :],
                                         AF.Identity, bias=db3v[0:mp, m:m + 1],
                                         scale=1.0 / W3SCALE)
                kedst = ze if s == 5 else ke[s]
                p = ps.tile([128, BH], F32, tag="ps", name="pe3")
                for g in range(2):
                    mmdr(p[0:45, :], w3e[:, g, :, 0:45],
                         h2e[:, g, :, :], g == 0, g == 1)
                nc.scalar.activation(kedst[0:45, :], p[0:45, :], AF.Identity,
                                     bias=db3e[0:45, 0:1], scale=1.0 / W3SCALE)

                # ---- tangent (only when this stage's trace matters) ----
                if B_TAB[s] != 0.0:
                    ttr_scale = float(DT) * float(B_TAB[s])
                    # voxel: h1 <- h1^2 <- (h1^2-1)*u1 (= -g1), u2, g2, trace
                    nc.scalar.activation(h1v[:, :, :, :], h1v[:, :, :, :], AF.Square)
                    nc.vector.scalar_tensor_tensor(h1v[:, :, :, :], h1v[:, :, :, :],
                                                   1.0, u1v[:, :],
                                                   ALU.subtract, ALU.mult)
                    u2p = []
                    for m in range(4):
                        ms = slice(m * 128, (m + 1) * 128)
                        p = ps.tile([128, BH], F32, tag="ps", name="pv4")
                        u2p.append(p)
                        for g in range(2):
                            mmdr(p[:, :], w2v[:, g, :, ms],
                                 h1v[:, g, :, :], g == 0, g == 1)
                    nc.scalar.activation(h2v[:, :, :, :], h2v[:, :, :, :], AF.Square)
                    for m in range(4):
                        nc.vector.scalar_tensor_tensor(h2v[:, m // 2, m % 2, :],
                                                       h2v[:, m // 2, m % 2, :], 1.0,
                                                       u2p[m][:, :], ALU.subtract,
                                                       ALU.mult)
                    nc.vector.scalar_tensor_tensor(
                        h2v[:, :, :, :], h2v[:, :, :, :], 1.0, m3v[:, :],
                        ALU.mult, ALU.mult, accum_out=outs[:, col_qv:col_qv + 1])
                    nc.vector.scalar_tensor_tensor(
                        outs[:, col_tv:col_tv + 1], outs[:, col_qv:col_qv + 1],
                        ttr_scale, outs[:, col_tv:col_tv + 1], ALU.mult, ALU.add)
                    # energy
                    nc.scalar.activation(h1e[:, :, :, :], h1e[:, :, :, :], AF.Square)
                    nc.vector.scalar_tensor_tensor(h1e[:, :, :, :], h1e[:, :, :, :],
                                                   1.0, u1e[:, :],
                                                   ALU.subtract, ALU.mult)
                    u2pe = []
                    for m in range(4):
                        ms = slice(m * 128, (m + 1) * 128)
                        p = ps.tile([128, BH], F32, tag="ps", name="pe4")
                        u2pe.append(p)
                        for g in range(2):
                            mmdr(p[:, :], w2e[:, g, :, ms],
                                 h1e[:, g, :, :], g == 0, g == 1)
                    nc.scalar.activation(h2e[:, :, :, :], h2e[:, :, :, :], AF.Square)
                    for m in range(4):
                        nc.vector.scalar_tensor_tensor(h2e[:, m // 2, m % 2, :],
                                                       h2e[:, m // 2, m % 2, :], 1.0,
                                                       u2pe[m][:, :], ALU.subtract,
                                                       ALU.mult)
                    nc.vector.scalar_tensor_tensor(
                        h2e[:, :, :, :], h2e[:, :, :, :], 1.0, m3e[:, :],
                        ALU.mult, ALU.mult, accum_out=outs[:, col_qe:col_qe + 1])
                    nc.vector.scalar_tensor_tensor(
                        outs[:, col_te:col_te + 1], outs[:, col_qe:col_qe + 1],
                        ttr_scale, outs[:, col_te:col_te + 1], ALU.mult, ALU.add)

            def step_body(half, iv, col_tv, col_te, col_qv, col_qe):
                if USE_FORI:
                    nc.vector.tensor_copy(out=tstep[:, :], in_=tg[:, ds(iv * 6, 6)])
                else:
                    i0 = iv * 6
                    nc.vector.tensor_copy(out=tstep[:, :], in_=tg[:, i0:i0 + 6])
                for s in range(6):
                    stage(half, iv, s, col_tv, col_te, col_qv, col_qe)
                    # accumulate the final-update term for k_s as soon as it
                    # exists (keeps the step tail off the critical path)
                    if s == 0:
                        nc.vector.tensor_scalar_mul(xacc[:, 0:W], kv[0][:, 0:W],
                                                    float(B_TAB[0]))
                        nc.vector.tensor_scalar_mul(xacce[0:45, :], ke[0][0:45, :],
                                                    float(B_TAB[0]))
                    elif s in (2, 3, 4):
                        bj = float(B_TAB[s])
                        nc.vector.scalar_tensor_tensor(
                            xacc[:, 0:W], kv[s][:, 0:W], bj, xacc[:, 0:W],
                            ALU.mult, ALU.add)
                        nc.vector.scalar_tensor_tensor(
                            xacce[0:45, :], ke[s][0:45, :], bj, xacce[0:45, :],
                            ALU.mult, ALU.add)
                # tail: xx += xacc + b5*k5   (k'_5 lives in zb1/ze1)
                b5 = float(B_TAB[5])
                nc.vector.scalar_tensor_tensor(
                    xacc[:, 0:W], zbufs[1][:, 0:W], b5, xacc[:, 0:W],
                    ALU.mult, ALU.add)
                nc.vector.tensor_add(out=xx[:, 0:W], in0=xx[:, 0:W],
                                     in1=xacc[:, 0:W])
                nc.vector.scalar_tensor_tensor(
                    xacce[0:45, :], zes[1][0:45, :], b5, xacce[0:45, :],
                    ALU.mult, ALU.add)
                nc.vector.tensor_add(out=xxe[0:45, :], in0=xxe[0:45, :],
                                     in1=xacce[0:45, :])

            def epilogue(half, col_zvA, col_zvB, col_ze):
                nc.scalar.activation(kv[0][:, 0:W], xx[:, 0:W], AF.Square,
                                     accum_out=outs[:, col_zvA:col_zvA + 1])
                nc.scalar.activation(ke[0][0:45, 0:BH], xxe[0:45, :], AF.Square,
                                     accum_out=outs[0:45, col_ze:col_ze + 1])

            def whole_pass():
                nc.vector.memset(outs[:, :], 0.0)
                for half in (0, 1):
                    c0 = half * 7
                    tc.strict_bb_all_engine_barrier()
                    prologue(half)
                    if USE_FORI:
                        with tc.For_i(0, N_STEPS, hint_engines=HINTS) as iv:
                            step_body(half, iv, c0 + 0, c0 + 1, c0 + 5, c0 + 6)
                    else:
                        for iv in range(N_STEPS):
                            step_body(half, iv, c0 + 0, c0 + 1, c0 + 5, c0 + 6)
                    epilogue(half, c0 + 2, c0 + 3, c0 + 4)

            if reps == 1:
                whole_pass()
            else:
                with tc.For_i(0, reps, hint_engines=HINTS):
                    whole_pass()

            nc.sync.dma_start(out=out_d, in_=outs[:, :])

    nc.compile()
    return nc


def _get_program(reps=1):
    key = f"nc{reps}"
    if key not in _CACHE:
        _CACHE[key] = _build_program(reps)
    return _CACHE[key]


# ----------------------------------------------------------------------------
# Host-side packing
# ----------------------------------------------------------------------------
def _group_feat(xT, ngroups, rows_total):
    """[F, Bh] feature-major -> [128, ngroups*Bh] grouped, zero padded."""
    F, Bh = xT.shape
    assert F == rows_total
    out = np.zeros((128, ngroups * Bh), np.float32)
    for g in range(ngroups):
        r0, r1 = g * 128, min((g + 1) * 128, F)
        if r0 >= F:
            break
        out[0:r1 - r0, g * Bh:g * Bh + Bh] = xT[r0:r1]
    return out


def _pack_weights(inputs):
    W1v = np.asarray(inputs["W1v"], np.float32)
    b1v = np.asarray(inputs["b1v"], np.float32)
    W2v = np.asarray(inputs["W2v"], np.float32)
    b2v = np.asarray(inputs["b2v"], np.float32)
    W3v = np.asarray(inputs["W3v"], np.float32)
    b3v = np.asarray(inputs["b3v"], np.float32)
    W1e = np.asarray(inputs["W1e"], np.float32)
    b1e = np.asarray(inputs["b1e"], np.float32)
    W2e = np.asarray(inputs["W2e"], np.float32)
    b2e = np.asarray(inputs["b2e"], np.float32)
    W3e = np.asarray(inputs["W3e"], np.float32)
    b3e = np.asarray(inputs["b3e"], np.float32)

    import ml_dtypes
    FP8 = ml_dtypes.float8_e4m3

    d = {}
    # k-tiles g0..g3 = x rows (504, zero padded), tail tile = [cond rows; b1]
    w1x = _group_feat(W1v[:D], 4, D)                # [128, 4*512]
    tail = np.zeros((128, 512), np.float32)
    tail[0:46] = W1v[D + 1:D + 47]                  # cond rows
    tail[46] = b1v
    d["w1v"] = np.concatenate([w1x, tail], axis=1)  # [128, 5*512]
    d["w2v"] = (_group_feat(W2SCALE * W2v, 4, 512)
                .reshape(128, 2, 2, 512).astype(FP8))
    w3p = np.zeros((128, 4, 512), np.float32)
    w3p[:, :, 0:504] = (_group_feat((DT * W3SCALE) * W3v, 4, 512)
                        .reshape(128, 4, 504))
    d["w3v"] = w3p.reshape(128, 2, 2, 512).astype(FP8)
    d["w3vt"] = _group_feat(np.ascontiguousarray(W3v.T), 4, 504)
    d["b2v"] = np.ascontiguousarray(b2v.reshape(4, 128).T)
    db3 = (DT * b3v).astype(np.float32)
    db3g = np.zeros((128, 4), np.float32)
    for m in range(4):
        r0, r1 = m * 128, min((m + 1) * 128, 504)
        db3g[0:r1 - r0, m] = db3[r0:r1]
    d["db3v"] = db3g
    d["w1tg"] = np.ascontiguousarray(W1v[D].reshape(4, 128).T)   # t row of W1v
    w1eaug = np.vstack([W1e, b1e[None, :]])         # [48, 512]
    d["w1e"] = np.ascontiguousarray(w1eaug)
    d["w2e"] = (_group_feat(W2SCALE * W2e, 4, 512)
                .reshape(128, 2, 2, 512).astype(FP8))
    w3ep = np.zeros((128, 4, 48), np.float32)
    w3ep[:, :, 0:45] = (_group_feat((DT * W3SCALE) * W3e, 4, 512)
                        .reshape(128, 4, 45))
    d["w3e"] = w3ep.reshape(128, 2, 2, 48).astype(FP8)
    d["w3et"] = np.ascontiguousarray(W3e.T)         # [45, 512]
    d["b2e"] = np.ascontiguousarray(b2e.reshape(4, 128).T)
    d["db3e"] = (DT * b3e).astype(np.float32)[:, None]
    d["w1teg"] = np.ascontiguousarray(W1e[E].reshape(4, 128).T)  # t row of W1e
    tv = np.zeros(60, np.float32)
    for n in range(N_STEPS):
        t0 = np.float32(1.0) + DT * np.float32(n)
        for s in range(6):
            tv[6 * n + s] = t0 + np.float32(C_TAB[s]) * DT
    d["tg"] = np.tile(tv[None, :], (128, 1)).astype(np.float32)
    return d


def _pack_core(inputs, wpack, core):
    voxel = np.asarray(inputs["voxel"], np.float32)[core * BC:(core + 1) * BC]
    energy = np.asarray(inputs["energy"], np.float32)[core * BC:(core + 1) * BC]
    cond = np.asarray(inputs["cond"], np.float32)[core * BC:(core + 1) * BC]
    eps_v = np.asarray(inputs["eps_v"], np.float32)[core * BC:(core + 1) * BC]
    eps_e = np.asarray(inputs["eps_e"], np.float32)[core * BC:(core + 1) * BC]

    m = dict(wpack)
    for h in (0, 1):
        sl = slice(h * BH, (h + 1) * BH)
        xT = np.ascontiguousarray(voxel[sl].T)       # [504, 512]
        evT = np.ascontiguousarray(eps_v[sl].T)
        m[f"xv{h}"] = _group_feat(xT, 4, D)
        m[f"ev{h}"] = _group_feat(evT, 4, D)
        condv = np.ascontiguousarray(
            np.concatenate([energy[sl], cond[sl]], axis=1).T)  # [46, 512]
        ztl = np.ones((47, BH), np.float32)
        ztl[0:46] = condv
        m[f"ztl{h}"] = ztl
        m[f"xe{h}"] = np.ascontiguousarray(energy[sl].T)
        m[f"ee{h}"] = np.ascontiguousarray(eps_e[sl].T)
        ce = np.ones((3, BH), np.float32)
        ce[0] = 0.0            # t lane (t enters via the tanh bias)
        ce[1] = cond[sl, 0]
        m[f"ce{h}"] = ce
    return m


# ----------------------------------------------------------------------------
# Entry point
# ----------------------------------------------------------------------------
def kernel(**inputs) -> np.ndarray:
    global LAST_RESULTS
    from concourse import bass_utils

    nc = _get_program()
    wpack = _pack_weights(inputs)
    in_maps = [_pack_core(inputs, wpack, c) for c in range(N_CORES)]
    res = bass_utils.run_bass_kernel_spmd(nc, in_maps, core_ids=list(range(N_CORES)))
    LAST_RESULTS = res

    total = np.zeros((128, 14), np.float64)
    for r in res.results:
        total += r["out"].astype(np.float64)
    trv = total[:, 0].sum() + total[:, 7].sum()
    tre = total[:, 1].sum() + total[:, 8].sum()
    zsv = (total[:, 2].sum() + total[:, 3].sum()
           + total[:, 9].sum() + total[:, 10].sum())
    zse = total[:, 4].sum() + total[:, 11].sum()

    mean_lp_v = (-0.5 * zsv + trv) / B_TOT - 0.5 * D * LOG2PI
    mean_lp_e = (-0.5 * zse + tre) / B_TOT - 0.5 * E * LOG2PI
    loss = -(mean_lp_v + mean_lp_e)
    return np.array(loss, dtype=np.float32)

